# revision 14
# baseline (speedup 1.0000x reference)
"""Multi-head attention (B=2, S=2048, D=2048, H=16, hd=128) on 8 TRN2 NeuronCores.

Sharding: data-parallel over batch (2) x tensor-parallel over head groups (4).
Core c handles batch c//4 and heads [4*(c%4), 4*(c%4)+4). Each core computes
q/k/v projections for its 512 features, RoPE, full attention over S for its 4
heads, and a partial output projection y_partial = attn_local @ wo[:, cols].T.
Host sums the 4 partials per batch (no on-chip collectives).

All matmuls run in f16 with fp32 PSUM accumulation. The 1/sqrt(hd) score
scale is folded into wq host-side. RoPE pairs are split even/odd across the
partition dim by permuting wq/wk rows host-side. Scores are computed
transposed ([k, q]) so softmax(exp)@V needs no on-chip transposes.

v2 changes vs v1 (trace-driven):
- x is DMA'd once into persistent SBUF (k/q/v all reuse it), staged in
  dc-quarters so the first k matmul starts ~4us in instead of ~20us.
- k/q projections run dc-major (4 open PSUM banks, one per head), so the
  first matmul only needs a quarter of wk and of x chunk 0.
- RoPE: one ScalarE PSUM->SBUF f16 copy per group, DVE math on f16 SBUF
  (halves DVE time and removes double PSUM reads next to the PE).
- Softmax denominator adds run on GpSimd (idle otherwise); the LAST block's
  partition-reduce uses a ones-matmul on the PE (~0.4us) instead of the
  4.3us GpSimd all-reduce that sat on the tail critical path.
- Output-projection PSUM->SBUF copies alternate ScalarE/DVE and write f16;
  y is f16 in DRAM (host accumulates in f32).
"""

import numpy as np

B = 2
S = 2048
D = 2048
H = 16
HD = 128
P = 128
N_CORES = 8
H_LOC = 4          # heads per core
F = H_LOC * HD     # local features = 512
NCH = 4            # n-chunks of 512 over S
CH = S // NCH      # 512
DCH = D // P       # 16 contraction chunks
NT = S // P        # 16 row tiles
DQ = 2             # dc-stage size (8 stages of 2 dc each)

_F16 = np.float16


def _build_program():
    import concourse.bass_isa as bass_isa
    import concourse.mybir as mybir
    import concourse.tile as tile
    from concourse import bacc

    dt = mybir.dt
    nc = bacc.Bacc("TRN2", target_bir_lowering=False, debug=False,
                   num_devices=N_CORES)

    # partition-major layouts so every DMA reads >=2KB contiguous per line
    xTc = nc.dram_tensor("xTc", [NCH, P, DCH, CH], dt.float16,
                         kind="ExternalInput").ap()
    wqT = nc.dram_tensor("wqT", [P, DCH, F], dt.float16,
                         kind="ExternalInput").ap()
    wkT = nc.dram_tensor("wkT", [P, DCH, F], dt.float16,
                         kind="ExternalInput").ap()
    wvT = nc.dram_tensor("wvT", [P, DCH, F], dt.float16,
                         kind="ExternalInput").ap()
    woT = nc.dram_tensor("woT", [P, H_LOC, D], dt.float16,
                         kind="ExternalInput").ap()
    # stacked RoPE tables: [cos;cos] and [sin;sin]
    ct = nc.dram_tensor("ct", [P, S], dt.float16, kind="ExternalInput").ap()
    st = nc.dram_tensor("st", [P, S], dt.float16, kind="ExternalInput").ap()
    y = nc.dram_tensor("y", [S, D], dt.float16, kind="ExternalOutput").ap()

    y3 = y.rearrange("(o p) n -> p o n", p=P)        # [128, 16, 2048]

    NB = NCH * H_LOC  # 16 attention blocks, b = qc*4 + h

    with tile.TileContext(nc) as tc:
        with tc.tile_pool(name="persist", bufs=1) as pp:
            qTp = pp.tile([P, H_LOC, S], dt.float16, tag="qTp")
            kTp = pp.tile([P, H_LOC, S], dt.float16, tag="kTp")
            v_sb = pp.tile([P, NT, F], dt.float16, tag="v")
            wo_sb = pp.tile([P, H_LOC, D], dt.float16, tag="wo")
            ones = pp.tile([P, P], dt.float16, tag="ones")
            # x resident for the whole kernel: chunks 0-1 as staged tiles
            # (fine-grained DMA deps so k-proj never waits), chunks 2-3 whole.
            NSTG = DCH // DQ
            xst = [[pp.tile([P, DQ, CH], dt.float16, tag=f"x{c}q{j}",
                            name=f"x{c}q{j}") for j in range(NSTG)]
                   for c in range(2)]
            xc_t = [None, None] + [
                pp.tile([P, DCH, CH], dt.float16, tag=f"xc{c}",
                        name=f"xc{c}") for c in range(2, NCH)]
            nc.vector.memset(ones[:], 1.0)
            warm = pp.tile([P, CH], dt.float16, tag="warm")
            nc.vector.memset(warm[:], 0.125)

            def xslice(c, dc, nsl=slice(None)):
                if c < 2:
                    return xst[c][dc // DQ][:, dc % DQ, nsl]
                return xc_t[c][:, dc, nsl]

            # ---- phase 1: k and q projections + RoPE ---------------------
            with (
                tc.tile_pool(name="wp", bufs=1) as wp,
                tc.tile_pool(name="t2p", bufs=1) as t2p,
                tc.tile_pool(name="psg", bufs=1, space="PSUM") as psg,
            ):
                wk_q = [wp.tile([P, DQ, F], dt.float16, tag=f"wk{j}",
                               name=f"wk{j}") for j in range(DCH // DQ)]
                wq_sb = wp.tile([P, DCH, F], dt.float16, tag="wq")
                ct_sb = wp.tile([P, S], dt.float16, tag="ct")
                st_sb = wp.tile([P, S], dt.float16, tag="st")

                # DMA issue order = need order: first stage of wk + first
                # stage of x chunk 0 gate the very first matmul; x chunk 1
                # ahead of the RoPE tables (first RoPE is ~28us in).
                for j in range(NSTG):
                    nc.sync.dma_start(wk_q[j][:], wkT[:, j * DQ:(j + 1) * DQ, :])
                    nc.sync.dma_start(xst[0][j][:],
                                      xTc[0, :, j * DQ:(j + 1) * DQ, :])
                for j in range(NSTG):
                    nc.sync.dma_start(xst[1][j][:],
                                      xTc[1, :, j * DQ:(j + 1) * DQ, :])
                nc.sync.dma_start(ct_sb[:], ct[:])
                nc.sync.dma_start(st_sb[:], st[:])
                for c in range(2, NCH):
                    nc.sync.dma_start(xc_t[c][:], xTc[c])
                nc.sync.dma_start(wq_sb[:], wqT[:])

                # ~30 dummy matmuls on memset data bridge the initial DMA
                # window so the PE p-state/HAM are fully ramped when the
                # first real matmul's operands land (~11us in).
                for w in range(30):
                    wps = psg.tile([P, CH], dt.float32, tag="gemm", bufs=8,
                                   name=f"warm{w}")
                    nc.tensor.matmul(wps[:], ones[:], warm[:],
                                     start=True, stop=True)

                def wkslice(dc, hsl):
                    return wk_q[dc // DQ][:, dc % DQ, hsl]

                def proj_chunk_rope(wsl, outT, c):
                    """One n-chunk of a projection, dc-major with 4 open
                    PSUM banks (one per head), then RoPE per head."""
                    nsl = slice(c * CH, (c + 1) * CH)
                    banks = [psg.tile([P, CH], dt.float32, tag="gemm",
                                      bufs=8, name=f"gemm{_h}")
                             for _h in range(H_LOC)]
                    for dc in range(DCH):
                        xs = xslice(c, dc)
                        for h in range(H_LOC):
                            nc.tensor.matmul(
                                banks[h][:], wsl(dc, slice(h * HD, (h + 1) * HD)),
                                xs, start=(dc == 0), stop=(dc == DCH - 1))
                    for h in range(H_LOC):
                        # RoPE: partitions 0:64 even pairs e, 64:128 odd o:
                        #   out_e = e*c - o*s ; out_o = e*s + o*c
                        # t2s holds the sin product with halves pre-swapped so
                        # every TensorTensor's two SBUF inputs share a base
                        # partition (walrus NCC_IBIR297).
                        s16 = t2p.tile([P, CH], dt.float16, tag="s16", bufs=3)
                        nc.scalar.activation(
                            s16[:], banks[h][:],
                            mybir.ActivationFunctionType.Copy)
                        t1 = t2p.tile([P, CH], dt.float16, tag="t1", bufs=2)
                        t2s = t2p.tile([P, CH], dt.float16, tag="t2", bufs=2)
                        nc.vector.tensor_mul(out=t1[:], in0=s16[:],
                                             in1=ct_sb[:, nsl])
                        nc.vector.tensor_mul(out=t2s[0:64, :],
                                             in0=s16[64:128, :],
                                             in1=st_sb[64:128, nsl])
                        nc.vector.tensor_mul(out=t2s[64:128, :],
                                             in0=s16[0:64, :],
                                             in1=st_sb[0:64, nsl])
                        o_sl = outT[:, h, nsl]
                        nc.vector.tensor_sub(out=o_sl[0:64, :],
                                             in0=t1[0:64, :],
                                             in1=t2s[0:64, :])
                        nc.vector.tensor_add(out=o_sl[64:128, :],
                                             in0=t1[64:128, :],
                                             in1=t2s[64:128, :])

                for c in range(NCH):
                    proj_chunk_rope(wkslice, kTp, c)
                for c in range(NCH):
                    proj_chunk_rope(
                        lambda dc, hsl: wq_sb[:, dc, hsl], qTp, c)

            # ---- phase 2: v proj + scores pipeline + pv + projection -----
            with (
                tc.tile_pool(name="etp", bufs=15) as etp,
                tc.tile_pool(name="attnp", bufs=2) as attnp,
                tc.tile_pool(name="accp", bufs=1) as accp,
                tc.tile_pool(name="ytp", bufs=3) as ytp,
                tc.tile_pool(name="psc", bufs=1, space="PSUM") as psc,
            ):
                acc_of = {}
                from collections import deque
                sc_iters = deque()

                def scores_gen(b):
                    """Emit one score+exp+acc unit (2 matmuls) per yield, so
                    callers can interleave units with other TensorE work."""
                    qc, h = divmod(b, H_LOC)
                    qsl = slice(qc * CH, (qc + 1) * CH)
                    ets = []
                    acc = accp.tile([P, 2, CH], dt.float16, tag="acc", bufs=3)
                    acc_of[b] = (acc, ets)
                    for ktp in range(NT // 2):
                        ss = psc.tile([P, 2, CH], dt.float32, tag="ss", bufs=2)
                        for i in range(2):
                            kt = 2 * ktp + i
                            nc.tensor.matmul(
                                ss[:, i, :], kTp[:, h, kt * P:(kt + 1) * P],
                                qTp[:, h, qsl], start=True, stop=True)
                        et = etp.tile([P, 2, CH], dt.float16, tag="et")
                        nc.scalar.activation(
                            et[:], ss[:], mybir.ActivationFunctionType.Exp)
                        if ktp == 0:
                            nc.vector.tensor_copy(acc[:], et[:])
                        else:
                            nc.vector.tensor_add(out=acc[:], in0=acc[:],
                                                 in1=et[:])
                        ets.append(et)
                        yield

                def pump(n=1):
                    for _ in range(n):
                        while sc_iters:
                            try:
                                next(sc_iters[0])
                                break
                            except StopIteration:
                                sc_iters.popleft()

                allr_of = {}

                def emit_allred(b):
                    # denominator add + all-reduce on GpSimd (idle engine),
                    # emitted one block ahead so its latency hides under the
                    # preceding pv matmuls.
                    acc, _ = acc_of[b]
                    denom = accp.tile([P, CH], dt.float32, tag="den", bufs=2)
                    nc.vector.tensor_add(out=denom[:], in0=acc[:, 0, :],
                                         in1=acc[:, 1, :])
                    allr = accp.tile([P, CH], dt.float32, tag="allr", bufs=2)
                    nc.gpsimd.partition_all_reduce(
                        allr[:], denom[:], channels=P,
                        reduce_op=bass_isa.ReduceOp.add)
                    allr_of[b] = allr

                def pv_block(b, attn_cur):
                    qc, h = divmod(b, H_LOC)
                    boundary = (h == H_LOC - 1)  # attn gates a proj chunk
                    hsl = slice(h * HD, (h + 1) * HD)
                    acc, ets = acc_of.pop(b)
                    pv = psc.tile([P, CH], dt.float32, tag="pv", bufs=2)
                    for ktp in range(NT // 2):
                        et = ets[ktp]
                        for i in range(2):
                            kt = 2 * ktp + i
                            nc.tensor.matmul(
                                pv[:], v_sb[:, kt, hsl], et[:, i, :],
                                start=(kt == 0), stop=(kt == NT - 1))
                        if not boundary:
                            pump(1)
                    rec = accp.tile([P, CH], dt.float32, tag="rec", bufs=2)
                    if boundary:
                        # chunk-last block: its attn gates the next proj
                        # chunk, so (a) partition-reduce the denominator with
                        # accumulating ones-matmuls on the PE instead of the
                        # 4.3us GpSimd all-reduce, and (b) defer the score
                        # pumps so the reciprocal/attn-mul sit at the DVE
                        # queue head. Cuts each boundary stall ~8us -> ~2us.
                        dps = psc.tile([P, CH], dt.float32, tag="py", bufs=2)
                        nc.tensor.matmul(dps[:], ones[:], acc[:, 0, :],
                                         start=True, stop=False)
                        nc.tensor.matmul(dps[:], ones[:], acc[:, 1, :],
                                         start=False, stop=True)
                        nc.vector.reciprocal_approx_fast(rec[:], dps[:])
                    else:
                        allr = allr_of.pop(b)
                        nc.vector.reciprocal_approx_fast(rec[:], allr[:])
                    nc.vector.tensor_mul(
                        out=attn_cur[:, h, :], in0=pv[:], in1=rec[:])
                    if b + 1 < NB and (b + 1) % H_LOC != H_LOC - 1:
                        emit_allred(b + 1)
                    if boundary:
                        pump(NT // 2)

                def proj_chunk(qc, attn_cur, ntls=range(NCH)):
                    for ntl in ntls:
                        nt = qc * NCH + ntl
                        for half in range(2):
                            yt = ytp.tile([P, D // 2], dt.float16, tag="yt")
                            for i in range(2):
                                oc = half * 2 + i
                                py = psc.tile([P, CH], dt.float32, tag="py",
                                              bufs=2)
                                for h in range(H_LOC):
                                    nc.tensor.matmul(
                                        py[:],
                                        attn_cur[:, h, ntl * P:(ntl + 1) * P],
                                        wo_sb[:, h, oc * CH:(oc + 1) * CH],
                                        start=(h == 0), stop=(h == H_LOC - 1))
                                ysl = yt[:, i * CH:(i + 1) * CH]
                                # all y copies on DVE: ScalarE stays
                                # exp-only in steady state (it paces the
                                # whole pipeline)
                                nc.vector.tensor_copy(ysl, py[:])
                            nc.sync.dma_start(
                                y3[:, nt, half * D // 2:(half + 1) * D // 2],
                                yt[:])
                            # keep score exp/acc units flowing between proj
                            # groups -- a burst of copies ahead of the exps
                            # stalls the next chunk boundary by ~10us
                            pump(1)

                # ---- v projection (x already resident), with the first two
                # score blocks pumped between v PSUM groups -----------------
                with tc.tile_pool(name="vwp", bufs=1) as vwp:
                    wv_sb = vwp.tile([P, DCH, F], dt.float16, tag="wv")
                    nc.sync.dma_start(wv_sb[:], wvT[:])
                    nc.sync.dma_start(wo_sb[:], woT[:])
                    sc_iters.append(scores_gen(0))
                    sc_iters.append(scores_gen(1))
                    for nchunk in range(NCH):
                        for nt in range(NCH):
                            ps = psc.tile([P, CH], dt.float32, tag="pv",
                                          bufs=2)
                            for dc in range(DCH):
                                nc.tensor.matmul(
                                    ps[:],
                                    xslice(nchunk, dc,
                                           slice(nt * P, (nt + 1) * P)),
                                    wv_sb[:, dc, :],
                                    start=(dc == 0), stop=(dc == DCH - 1))
                            nc.scalar.activation(
                                v_sb[:, nchunk * NCH + nt, :], ps[:],
                                mybir.ActivationFunctionType.Copy)
                            pump(1)

                    # steady state: [pv(b) | scores(b+2) units | proj(qc-1)]
                    attn_hist = {}
                    emit_allred(0)
                    for b in range(NB):
                        qc = b // H_LOC
                        if b % H_LOC == 0:
                            attn_hist[qc] = attnp.tile(
                                [P, H_LOC, CH], dt.float16, tag="attn",
                                name=f"attn_{qc}")
                        if b + 2 < NB:
                            sc_iters.append(scores_gen(b + 2))
                        pv_block(b, attn_hist[qc])
                        if b % H_LOC == 0 and b > 0:
                            proj_chunk(qc - 1, attn_hist.pop(qc - 1))
                    pump(100)
                    proj_chunk(NCH - 1, attn_hist.pop(NCH - 1))

    nc.compile()
    return nc


_NC_CACHE = None


def _get_program():
    global _NC_CACHE
    if _NC_CACHE is None:
        _NC_CACHE = _build_program()
    return _NC_CACHE


def _rope_tables():
    scale = np.arange(0, HD, 2, dtype=np.float32) / HD
    inv_freq = 1.0 / (10000.0 ** scale)                 # [64]
    t = np.arange(S, dtype=np.float32)
    ang = np.outer(t, inv_freq)                         # [S, 64]
    cos = np.cos(ang).T.astype(np.float32)              # [64, S]
    sin = np.sin(ang).T.astype(np.float32)
    stk = lambda a: np.ascontiguousarray(
        np.concatenate([a, a], axis=0)).astype(_F16)    # [128, S]
    return stk(cos), stk(sin)


def prepare_in_maps(x, wq, wk, wv, wo):
    x = np.asarray(x, dtype=np.float32)
    wq = np.asarray(wq, dtype=np.float32) * np.float32(1.0 / np.sqrt(HD))
    wk = np.asarray(wk, dtype=np.float32)
    wv = np.asarray(wv, dtype=np.float32)
    wo = np.asarray(wo, dtype=np.float32)

    ct_t, st_t = _rope_tables()

    # even/odd RoPE permutation of rows within each head
    perm = np.concatenate([np.arange(0, HD, 2), np.arange(1, HD, 2)])

    # [NCH, P, DCH, CH]: per-partition-contiguous x chunks
    xTc = [np.ascontiguousarray(
        x[b].T.reshape(DCH, P, NCH, CH).transpose(2, 1, 0, 3)).astype(_F16)
        for b in range(B)]

    in_maps = []
    for c in range(N_CORES):
        b, hg = divmod(c, H_LOC)
        heads = np.arange(hg * H_LOC, (hg + 1) * H_LOC)
        rows_qk = (heads[:, None] * HD + perm[None, :]).reshape(-1)  # [512]
        rows_nat = np.arange(hg * F, (hg + 1) * F)
        def pmaj(wT, groups):  # [D_in, F] -> [P, groups, F]
            return np.ascontiguousarray(
                wT.reshape(groups, P, wT.shape[1]).transpose(1, 0, 2)
            ).astype(_F16)
        in_maps.append({
            "xTc": xTc[b],
            "wqT": pmaj(wq[rows_qk].T, DCH),
            "wkT": pmaj(wk[rows_qk].T, DCH),
            "wvT": pmaj(wv[rows_nat].T, DCH),
            "woT": pmaj(wo[:, rows_nat].T, H_LOC),
            "ct": ct_t, "st": st_t,
        })
    return in_maps


def combine_results(results):
    out = np.zeros((B, S, D), dtype=np.float32)
    for c, r in enumerate(results):
        out[c // H_LOC] += r["y"].astype(np.float32)
    return out


def kernel(x, wq, wk, wv, wo):
    from concourse.bass_utils import run_bass_kernel_spmd

    nc = _get_program()
    in_maps = prepare_in_maps(x, wq, wk, wv, wo)
    res = run_bass_kernel_spmd(nc, in_maps, core_ids=list(range(N_CORES)))
    return combine_results(res.results)


if __name__ == "__main__":
    rng = np.random.default_rng(0)
    ins = {
        "x": rng.standard_normal((B, S, D), dtype=np.float32),
        "wq": rng.standard_normal((D, D), dtype=np.float32) / np.sqrt(D),
        "wk": rng.standard_normal((D, D), dtype=np.float32) / np.sqrt(D),
        "wv": rng.standard_normal((D, D), dtype=np.float32) / np.sqrt(D),
        "wo": rng.standard_normal((D, D), dtype=np.float32) / np.sqrt(D),
    }
    out = kernel(**ins)
    print("out", out.shape, out.dtype, np.abs(out).max())


# revision 16
# speedup vs baseline: 1.1370x; 1.1370x over previous
"""Multi-head attention (B=2, S=2048, D=2048, H=16, hd=128) on 8 TRN2 NeuronCores.

Sharding: data-parallel over batch (2) x tensor-parallel over head groups (4).
Core c handles batch c//4 and heads [4*(c%4), 4*(c%4)+4). Each core computes
q/k/v projections for its 512 features, RoPE, full attention over S for its 4
heads, and a partial output projection y_partial = attn_local @ wo[:, cols].T.
Host sums the 4 partials per batch (no on-chip collectives).

All matmuls run in f16 with fp32 PSUM accumulation. The 1/sqrt(hd) score
scale is folded into wq host-side. RoPE pairs are split even/odd across the
partition dim by permuting wq/wk rows host-side. Scores are computed
transposed ([k, q]) so softmax(exp)@V needs no on-chip transposes.

Trace-driven optimizations vs the original pipeline (457us -> ~394us):
- x is DMA'd once into persistent SBUF (k/q/v all reuse it); wk and x
  chunks 0-1 are staged in 2-dc tiles so the first k matmul starts ~11us
  in (runtime preamble + 1MB DMA) instead of ~20us.
- k/q projections run dc-major (4 open PSUM banks, one per head), so the
  first matmul only needs the first stage of wk and of x chunk 0.
- RoPE: one ScalarE PSUM->SBUF f16 copy per group, then DVE math on f16
  SBUF with the sin-product halves pre-swapped (walrus requires equal
  base partitions for SBUF+SBUF TensorTensor inputs).
- Chunk-last blocks (attn gates the next output-projection chunk):
  the softmax denominator partition-reduce runs as accumulating
  ones-matmuls on the PE (~0.4us) instead of the 4.3us GpSimd
  all-reduce, and the score pumps are deferred so reciprocal/attn-mul
  sit at the DVE queue head. Other blocks keep the GpSimd all-reduce
  (fully hidden under pv matmuls).
- proj_chunk pumps one score exp/acc unit per PSUM group: a burst of
  PSUM->SBUF copies enqueued ahead of the exps otherwise stalls the
  next chunk boundary ~10us and drops the PE out of its top p-state
  (matmul spacing degrades 216ns -> 259ns kernel-wide).
- Output-projection copies split ScalarE/DVE; y is f16 in DRAM (host
  accumulates in f32).
"""

import numpy as np

B = 2
S = 2048
D = 2048
H = 16
HD = 128
P = 128
N_CORES = 8
H_LOC = 4          # heads per core
F = H_LOC * HD     # local features = 512
NCH = 4            # n-chunks of 512 over S
CH = S // NCH      # 512
DCH = D // P       # 16 contraction chunks
NT = S // P        # 16 row tiles
DQ = 2             # dc-stage size (8 stages of 2 dc each)

_F16 = np.float16


def _build_program():
    import concourse.bass_isa as bass_isa
    import concourse.mybir as mybir
    import concourse.tile as tile
    from concourse import bacc

    dt = mybir.dt
    nc = bacc.Bacc("TRN2", target_bir_lowering=False, debug=False,
                   num_devices=N_CORES)

    # partition-major layouts so every DMA reads >=2KB contiguous per line
    xTc = nc.dram_tensor("xTc", [NCH, P, DCH, CH], dt.float16,
                         kind="ExternalInput").ap()
    wqT = nc.dram_tensor("wqT", [P, DCH, F], dt.float16,
                         kind="ExternalInput").ap()
    wkT = nc.dram_tensor("wkT", [P, DCH, F], dt.float16,
                         kind="ExternalInput").ap()
    wvT = nc.dram_tensor("wvT", [P, DCH, F], dt.float16,
                         kind="ExternalInput").ap()
    woT = nc.dram_tensor("woT", [P, H_LOC, D], dt.float16,
                         kind="ExternalInput").ap()
    # stacked RoPE tables: [cos;cos] and [sin;sin]
    ct = nc.dram_tensor("ct", [P, S], dt.float16, kind="ExternalInput").ap()
    st = nc.dram_tensor("st", [P, S], dt.float16, kind="ExternalInput").ap()
    y = nc.dram_tensor("y", [S, D], dt.float16, kind="ExternalOutput").ap()

    y3 = y.rearrange("(o p) n -> p o n", p=P)        # [128, 16, 2048]

    NB = NCH * H_LOC  # 16 attention blocks, b = qc*4 + h

    with tile.TileContext(nc) as tc:
        with tc.tile_pool(name="persist", bufs=1) as pp:
            qTp = pp.tile([P, H_LOC, S], dt.float16, tag="qTp")
            kTp = pp.tile([P, H_LOC, S], dt.float16, tag="kTp")
            v_sb = pp.tile([P, NT, F], dt.float16, tag="v")
            wo_sb = pp.tile([P, H_LOC, D], dt.float16, tag="wo")
            ones = pp.tile([P, P], dt.float16, tag="ones")
            # x resident for the whole kernel: chunks 0-1 as staged tiles
            # (fine-grained DMA deps so k-proj never waits), chunks 2-3 whole.
            NSTG = DCH // DQ
            xst = [[pp.tile([P, DQ, CH], dt.float16, tag=f"x{c}q{j}",
                            name=f"x{c}q{j}") for j in range(NSTG)]
                   for c in range(2)]
            xc_t = [None, None] + [
                pp.tile([P, DCH, CH], dt.float16, tag=f"xc{c}",
                        name=f"xc{c}") for c in range(2, NCH)]
            nc.vector.memset(ones[:], 1.0)

            def xslice(c, dc, nsl=slice(None)):
                if c < 2:
                    return xst[c][dc // DQ][:, dc % DQ, nsl]
                return xc_t[c][:, dc, nsl]

            # ---- phase 1: k and q projections + RoPE ---------------------
            with (
                tc.tile_pool(name="wp", bufs=1) as wp,
                tc.tile_pool(name="t2p", bufs=1) as t2p,
                tc.tile_pool(name="psg", bufs=1, space="PSUM") as psg,
            ):
                wk_q = [wp.tile([P, DQ, F], dt.float16, tag=f"wk{j}",
                               name=f"wk{j}") for j in range(DCH // DQ)]
                wq_sb = wp.tile([P, DCH, F], dt.float16, tag="wq")
                ct_sb = wp.tile([P, S], dt.float16, tag="ct")
                st_sb = wp.tile([P, S], dt.float16, tag="st")

                # DMA issue order = need order: first stage of wk + first
                # stage of x chunk 0 gate the very first matmul; x chunk 1
                # ahead of the RoPE tables (first RoPE is ~28us in).
                for j in range(NSTG):
                    nc.sync.dma_start(wk_q[j][:], wkT[:, j * DQ:(j + 1) * DQ, :])
                    nc.sync.dma_start(xst[0][j][:],
                                      xTc[0, :, j * DQ:(j + 1) * DQ, :])
                for j in range(NSTG):
                    nc.sync.dma_start(xst[1][j][:],
                                      xTc[1, :, j * DQ:(j + 1) * DQ, :])
                nc.sync.dma_start(ct_sb[:], ct[:])
                nc.sync.dma_start(st_sb[:], st[:])
                for c in range(2, NCH):
                    nc.sync.dma_start(xc_t[c][:], xTc[c])
                nc.sync.dma_start(wq_sb[:], wqT[:])

                def wkslice(dc, hsl):
                    return wk_q[dc // DQ][:, dc % DQ, hsl]

                def proj_chunk_rope(wsl, outT, c):
                    """One n-chunk of a projection, dc-major with 4 open
                    PSUM banks (one per head), then RoPE per head."""
                    nsl = slice(c * CH, (c + 1) * CH)
                    banks = [psg.tile([P, CH], dt.float32, tag="gemm",
                                      bufs=8, name=f"gemm{_h}")
                             for _h in range(H_LOC)]
                    for dc in range(DCH):
                        xs = xslice(c, dc)
                        for h in range(H_LOC):
                            nc.tensor.matmul(
                                banks[h][:], wsl(dc, slice(h * HD, (h + 1) * HD)),
                                xs, start=(dc == 0), stop=(dc == DCH - 1))
                    for h in range(H_LOC):
                        # RoPE: partitions 0:64 even pairs e, 64:128 odd o:
                        #   out_e = e*c - o*s ; out_o = e*s + o*c
                        # t2s holds the sin product with halves pre-swapped so
                        # every TensorTensor's two SBUF inputs share a base
                        # partition (walrus NCC_IBIR297).
                        s16 = t2p.tile([P, CH], dt.float16, tag="s16", bufs=3)
                        nc.scalar.activation(
                            s16[:], banks[h][:],
                            mybir.ActivationFunctionType.Copy)
                        t1 = t2p.tile([P, CH], dt.float16, tag="t1", bufs=2)
                        t2s = t2p.tile([P, CH], dt.float16, tag="t2", bufs=2)
                        nc.vector.tensor_mul(out=t1[:], in0=s16[:],
                                             in1=ct_sb[:, nsl])
                        nc.vector.tensor_mul(out=t2s[0:64, :],
                                             in0=s16[64:128, :],
                                             in1=st_sb[64:128, nsl])
                        nc.vector.tensor_mul(out=t2s[64:128, :],
                                             in0=s16[0:64, :],
                                             in1=st_sb[0:64, nsl])
                        o_sl = outT[:, h, nsl]
                        nc.vector.tensor_sub(out=o_sl[0:64, :],
                                             in0=t1[0:64, :],
                                             in1=t2s[0:64, :])
                        nc.vector.tensor_add(out=o_sl[64:128, :],
                                             in0=t1[64:128, :],
                                             in1=t2s[64:128, :])

                for c in range(NCH):
                    proj_chunk_rope(wkslice, kTp, c)
                for c in range(NCH):
                    proj_chunk_rope(
                        lambda dc, hsl: wq_sb[:, dc, hsl], qTp, c)

            # ---- phase 2: v proj + scores pipeline + pv + projection -----
            with (
                tc.tile_pool(name="etp", bufs=14) as etp,
                tc.tile_pool(name="attnp", bufs=2) as attnp,
                tc.tile_pool(name="accp", bufs=1) as accp,
                tc.tile_pool(name="ytp", bufs=4) as ytp,
                tc.tile_pool(name="psc", bufs=1, space="PSUM") as psc,
            ):
                acc_of = {}
                from collections import deque
                sc_iters = deque()

                def scores_gen(b):
                    """Emit one score+exp+acc unit (2 matmuls) per yield, so
                    callers can interleave units with other TensorE work."""
                    qc, h = divmod(b, H_LOC)
                    qsl = slice(qc * CH, (qc + 1) * CH)
                    ets = []
                    acc = accp.tile([P, 2, CH], dt.float16, tag="acc", bufs=3)
                    acc_of[b] = (acc, ets)
                    for ktp in range(NT // 2):
                        ss = psc.tile([P, 2, CH], dt.float32, tag="ss", bufs=2)
                        for i in range(2):
                            kt = 2 * ktp + i
                            nc.tensor.matmul(
                                ss[:, i, :], kTp[:, h, kt * P:(kt + 1) * P],
                                qTp[:, h, qsl], start=True, stop=True)
                        et = etp.tile([P, 2, CH], dt.float16, tag="et")
                        nc.scalar.activation(
                            et[:], ss[:], mybir.ActivationFunctionType.Exp)
                        if ktp == 0:
                            nc.vector.tensor_copy(acc[:], et[:])
                        else:
                            nc.vector.tensor_add(out=acc[:], in0=acc[:],
                                                 in1=et[:])
                        ets.append(et)
                        yield

                def pump(n=1):
                    for _ in range(n):
                        while sc_iters:
                            try:
                                next(sc_iters[0])
                                break
                            except StopIteration:
                                sc_iters.popleft()

                allr_of = {}

                def emit_allred(b):
                    # denominator add + all-reduce on GpSimd (idle engine),
                    # emitted one block ahead so its latency hides under the
                    # preceding pv matmuls.
                    acc, _ = acc_of[b]
                    denom = accp.tile([P, CH], dt.float32, tag="den", bufs=2)
                    nc.vector.tensor_add(out=denom[:], in0=acc[:, 0, :],
                                         in1=acc[:, 1, :])
                    allr = accp.tile([P, CH], dt.float32, tag="allr", bufs=2)
                    nc.gpsimd.partition_all_reduce(
                        allr[:], denom[:], channels=P,
                        reduce_op=bass_isa.ReduceOp.add)
                    allr_of[b] = allr

                def pv_block(b, attn_cur):
                    qc, h = divmod(b, H_LOC)
                    boundary = (h == H_LOC - 1)  # attn gates a proj chunk
                    hsl = slice(h * HD, (h + 1) * HD)
                    acc, ets = acc_of.pop(b)
                    pv = psc.tile([P, CH], dt.float32, tag="pv", bufs=2)
                    for ktp in range(NT // 2):
                        et = ets[ktp]
                        for i in range(2):
                            kt = 2 * ktp + i
                            nc.tensor.matmul(
                                pv[:], v_sb[:, kt, hsl], et[:, i, :],
                                start=(kt == 0), stop=(kt == NT - 1))
                        if not boundary:
                            pump(1)
                    rec = accp.tile([P, CH], dt.float32, tag="rec", bufs=2)
                    if boundary:
                        # chunk-last block: its attn gates the next proj
                        # chunk, so (a) partition-reduce the denominator with
                        # accumulating ones-matmuls on the PE instead of the
                        # 4.3us GpSimd all-reduce, and (b) defer the score
                        # pumps so the reciprocal/attn-mul sit at the DVE
                        # queue head. Cuts each boundary stall ~8us -> ~2us.
                        dps = psc.tile([P, CH], dt.float32, tag="py", bufs=2)
                        nc.tensor.matmul(dps[:], ones[:], acc[:, 0, :],
                                         start=True, stop=False)
                        nc.tensor.matmul(dps[:], ones[:], acc[:, 1, :],
                                         start=False, stop=True)
                        nc.vector.reciprocal_approx_fast(rec[:], dps[:])
                    else:
                        allr = allr_of.pop(b)
                        nc.vector.reciprocal_approx_fast(rec[:], allr[:])
                    nc.vector.tensor_mul(
                        out=attn_cur[:, h, :], in0=pv[:], in1=rec[:])
                    if b + 1 < NB and (b + 1) % H_LOC != H_LOC - 1:
                        emit_allred(b + 1)
                    if boundary:
                        pump(NT // 2)

                def proj_chunk(qc, attn_cur, ntls=range(NCH)):
                    for ntl in ntls:
                        nt = qc * NCH + ntl
                        for half in range(2):
                            yt = ytp.tile([P, D // 2], dt.float16, tag="yt")
                            for i in range(2):
                                oc = half * 2 + i
                                py = psc.tile([P, CH], dt.float32, tag="py",
                                              bufs=2)
                                for h in range(H_LOC):
                                    nc.tensor.matmul(
                                        py[:],
                                        attn_cur[:, h, ntl * P:(ntl + 1) * P],
                                        wo_sb[:, h, oc * CH:(oc + 1) * CH],
                                        start=(h == 0), stop=(h == H_LOC - 1))
                                ysl = yt[:, i * CH:(i + 1) * CH]
                                # split PSUM->SBUF copies across ScalarE and
                                # DVE so neither queue delays the score exps
                                if half == 0:
                                    nc.scalar.activation(
                                        ysl, py[:],
                                        mybir.ActivationFunctionType.Copy)
                                else:
                                    nc.vector.tensor_copy(ysl, py[:])
                            nc.sync.dma_start(
                                y3[:, nt, half * D // 2:(half + 1) * D // 2],
                                yt[:])
                            # keep score exp/acc units flowing between proj
                            # groups -- a burst of copies ahead of the exps
                            # stalls the next chunk boundary by ~10us
                            pump(1)

                # ---- v projection (x already resident), with the first two
                # score blocks pumped between v PSUM groups -----------------
                with tc.tile_pool(name="vwp", bufs=1) as vwp:
                    wv_sb = vwp.tile([P, DCH, F], dt.float16, tag="wv")
                    nc.sync.dma_start(wv_sb[:], wvT[:])
                    nc.sync.dma_start(wo_sb[:], woT[:])
                    sc_iters.append(scores_gen(0))
                    sc_iters.append(scores_gen(1))
                    for nchunk in range(NCH):
                        for nt in range(NCH):
                            ps = psc.tile([P, CH], dt.float32, tag="pv",
                                          bufs=2)
                            for dc in range(DCH):
                                nc.tensor.matmul(
                                    ps[:],
                                    xslice(nchunk, dc,
                                           slice(nt * P, (nt + 1) * P)),
                                    wv_sb[:, dc, :],
                                    start=(dc == 0), stop=(dc == DCH - 1))
                            nc.scalar.activation(
                                v_sb[:, nchunk * NCH + nt, :], ps[:],
                                mybir.ActivationFunctionType.Copy)
                            pump(1)

                    # steady state: [pv(b) | scores(b+2) units | proj(qc-1)]
                    attn_hist = {}
                    emit_allred(0)
                    for b in range(NB):
                        qc = b // H_LOC
                        if b % H_LOC == 0:
                            attn_hist[qc] = attnp.tile(
                                [P, H_LOC, CH], dt.float16, tag="attn",
                                name=f"attn_{qc}")
                        if b + 2 < NB:
                            sc_iters.append(scores_gen(b + 2))
                        pv_block(b, attn_hist[qc])
                        if b % H_LOC == 0 and b > 0:
                            proj_chunk(qc - 1, attn_hist.pop(qc - 1))
                    pump(100)
                    proj_chunk(NCH - 1, attn_hist.pop(NCH - 1))

    nc.compile()
    return nc


_NC_CACHE = None


def _get_program():
    global _NC_CACHE
    if _NC_CACHE is None:
        _NC_CACHE = _build_program()
    return _NC_CACHE


def _rope_tables():
    scale = np.arange(0, HD, 2, dtype=np.float32) / HD
    inv_freq = 1.0 / (10000.0 ** scale)                 # [64]
    t = np.arange(S, dtype=np.float32)
    ang = np.outer(t, inv_freq)                         # [S, 64]
    cos = np.cos(ang).T.astype(np.float32)              # [64, S]
    sin = np.sin(ang).T.astype(np.float32)
    stk = lambda a: np.ascontiguousarray(
        np.concatenate([a, a], axis=0)).astype(_F16)    # [128, S]
    return stk(cos), stk(sin)


def prepare_in_maps(x, wq, wk, wv, wo):
    x = np.asarray(x, dtype=np.float32)
    wq = np.asarray(wq, dtype=np.float32) * np.float32(1.0 / np.sqrt(HD))
    wk = np.asarray(wk, dtype=np.float32)
    wv = np.asarray(wv, dtype=np.float32)
    wo = np.asarray(wo, dtype=np.float32)

    ct_t, st_t = _rope_tables()

    # even/odd RoPE permutation of rows within each head
    perm = np.concatenate([np.arange(0, HD, 2), np.arange(1, HD, 2)])

    # [NCH, P, DCH, CH]: per-partition-contiguous x chunks
    xTc = [np.ascontiguousarray(
        x[b].T.reshape(DCH, P, NCH, CH).transpose(2, 1, 0, 3)).astype(_F16)
        for b in range(B)]

    in_maps = []
    for c in range(N_CORES):
        b, hg = divmod(c, H_LOC)
        heads = np.arange(hg * H_LOC, (hg + 1) * H_LOC)
        rows_qk = (heads[:, None] * HD + perm[None, :]).reshape(-1)  # [512]
        rows_nat = np.arange(hg * F, (hg + 1) * F)
        def pmaj(wT, groups):  # [D_in, F] -> [P, groups, F]
            return np.ascontiguousarray(
                wT.reshape(groups, P, wT.shape[1]).transpose(1, 0, 2)
            ).astype(_F16)
        in_maps.append({
            "xTc": xTc[b],
            "wqT": pmaj(wq[rows_qk].T, DCH),
            "wkT": pmaj(wk[rows_qk].T, DCH),
            "wvT": pmaj(wv[rows_nat].T, DCH),
            "woT": pmaj(wo[:, rows_nat].T, H_LOC),
            "ct": ct_t, "st": st_t,
        })
    return in_maps


def combine_results(results):
    out = np.zeros((B, S, D), dtype=np.float32)
    for c, r in enumerate(results):
        out[c // H_LOC] += r["y"].astype(np.float32)
    return out


def kernel(x, wq, wk, wv, wo):
    from concourse.bass_utils import run_bass_kernel_spmd

    nc = _get_program()
    in_maps = prepare_in_maps(x, wq, wk, wv, wo)
    res = run_bass_kernel_spmd(nc, in_maps, core_ids=list(range(N_CORES)))
    return combine_results(res.results)


if __name__ == "__main__":
    rng = np.random.default_rng(0)
    ins = {
        "x": rng.standard_normal((B, S, D), dtype=np.float32),
        "wq": rng.standard_normal((D, D), dtype=np.float32) / np.sqrt(D),
        "wk": rng.standard_normal((D, D), dtype=np.float32) / np.sqrt(D),
        "wv": rng.standard_normal((D, D), dtype=np.float32) / np.sqrt(D),
        "wo": rng.standard_normal((D, D), dtype=np.float32) / np.sqrt(D),
    }
    out = kernel(**ins)
    print("out", out.shape, out.dtype, np.abs(out).max())


# revision 18
# speedup vs baseline: 1.1911x; 1.0476x over previous
"""Multi-head attention (B=2, S=2048, D=2048, H=16, hd=128) on 8 TRN2 NeuronCores.

Sharding: data-parallel over batch (2) x tensor-parallel over head groups (4).
Core c handles batch c//4 and heads [4*(c%4), 4*(c%4)+4). Each core computes
q/k/v projections for its 512 features, RoPE, full attention over S for its 4
heads, and a partial output projection y_partial = attn_local @ wo[:, cols].T.
Host sums the 4 partials per batch (no on-chip collectives).

All matmuls run in f16 with fp32 PSUM accumulation. The 1/sqrt(hd) score
scale is folded into wq host-side. RoPE pairs are split even/odd across the
partition dim by permuting wq/wk rows host-side. Scores are computed
transposed ([k, q]) so softmax(exp)@V needs no on-chip transposes.

Trace-driven optimizations vs the original pipeline (457us -> ~394us):
- x is DMA'd once into persistent SBUF (k/q/v all reuse it); wk and x
  chunks 0-1 are staged in 2-dc tiles so the first k matmul starts ~11us
  in (runtime preamble + 1MB DMA) instead of ~20us.
- k/q projections run dc-major (4 open PSUM banks, one per head), so the
  first matmul only needs the first stage of wk and of x chunk 0.
- RoPE: one ScalarE PSUM->SBUF f16 copy per group, then DVE math on f16
  SBUF with the sin-product halves pre-swapped (walrus requires equal
  base partitions for SBUF+SBUF TensorTensor inputs).
- Chunk-last blocks (attn gates the next output-projection chunk):
  the softmax denominator partition-reduce runs as accumulating
  ones-matmuls on the PE (~0.4us) instead of the 4.3us GpSimd
  all-reduce, and the score pumps are deferred so reciprocal/attn-mul
  sit at the DVE queue head. Other blocks keep the GpSimd all-reduce
  (fully hidden under pv matmuls).
- proj_chunk pumps one score exp/acc unit per PSUM group: a burst of
  PSUM->SBUF copies enqueued ahead of the exps otherwise stalls the
  next chunk boundary ~10us and drops the PE out of its top p-state
  (matmul spacing degrades 216ns -> 259ns kernel-wide).
- Output-projection copies split ScalarE/DVE; y is f16 in DRAM (host
  accumulates in f32).
"""

import numpy as np

B = 2
S = 2048
D = 2048
H = 16
HD = 128
P = 128
N_CORES = 8
H_LOC = 4          # heads per core
F = H_LOC * HD     # local features = 512
NCH = 4            # n-chunks of 512 over S
CH = S // NCH      # 512
DCH = D // P       # 16 contraction chunks
NT = S // P        # 16 row tiles
DQ = 2             # dc-stage size (8 stages of 2 dc each)

_F16 = np.float16


def _build_program():
    import concourse.bass_isa as bass_isa
    import concourse.mybir as mybir
    import concourse.tile as tile
    from concourse import bacc

    dt = mybir.dt
    nc = bacc.Bacc("TRN2", target_bir_lowering=False, debug=False,
                   num_devices=N_CORES)

    # partition-major layouts so every DMA reads >=2KB contiguous per line
    xTc = nc.dram_tensor("xTc", [NCH, P, DCH, CH], dt.float16,
                         kind="ExternalInput").ap()
    wqT = nc.dram_tensor("wqT", [P, DCH, F], dt.float16,
                         kind="ExternalInput").ap()
    wkT = nc.dram_tensor("wkT", [P, DCH, F], dt.float16,
                         kind="ExternalInput").ap()
    wvT = nc.dram_tensor("wvT", [P, DCH, F], dt.float16,
                         kind="ExternalInput").ap()
    woT = nc.dram_tensor("woT", [P, H_LOC, D], dt.float16,
                         kind="ExternalInput").ap()
    # stacked RoPE tables: [cos;cos] and [sin;sin]
    ct = nc.dram_tensor("ct", [P, S], dt.float16, kind="ExternalInput").ap()
    st = nc.dram_tensor("st", [P, S], dt.float16, kind="ExternalInput").ap()
    y = nc.dram_tensor("y", [S, D], dt.float16, kind="ExternalOutput").ap()

    y3 = y.rearrange("(o p) n -> p o n", p=P)        # [128, 16, 2048]

    NB = NCH * H_LOC  # 16 attention blocks, b = qc*4 + h

    with tile.TileContext(nc) as tc:
        with tc.tile_pool(name="persist", bufs=1) as pp:
            qTp = pp.tile([P, H_LOC, S], dt.float16, tag="qTp")
            kTp = pp.tile([P, H_LOC, S], dt.float16, tag="kTp")
            v_sb = pp.tile([P, NT, F], dt.float16, tag="v")
            wo_sb = pp.tile([P, H_LOC, D], dt.float16, tag="wo")
            ones = pp.tile([P, P], dt.float16, tag="ones")
            # x resident for the whole kernel: chunks 0-1 as staged tiles
            # (fine-grained DMA deps so k-proj never waits), chunks 2-3 whole.
            NSTG = DCH // DQ
            xst = [[pp.tile([P, DQ, CH], dt.float16, tag=f"x{c}q{j}",
                            name=f"x{c}q{j}") for j in range(NSTG)]
                   for c in range(2)]
            xc_t = [None, None] + [
                pp.tile([P, DCH, CH], dt.float16, tag=f"xc{c}",
                        name=f"xc{c}") for c in range(2, NCH)]
            nc.vector.memset(ones[:], 1.0)

            def xslice(c, dc, nsl=slice(None)):
                if c < 2:
                    return xst[c][dc // DQ][:, dc % DQ, nsl]
                return xc_t[c][:, dc, nsl]

            # ---- phase 1: k and q projections + RoPE ---------------------
            with (
                tc.tile_pool(name="wp", bufs=1) as wp,
                tc.tile_pool(name="t2p", bufs=1) as t2p,
                tc.tile_pool(name="psg", bufs=1, space="PSUM") as psg,
            ):
                wk_q = [wp.tile([P, DQ, F], dt.float16, tag=f"wk{j}",
                               name=f"wk{j}") for j in range(DCH // DQ)]
                wq_sb = wp.tile([P, DCH, F], dt.float16, tag="wq")
                ct_sb = wp.tile([P, S], dt.float16, tag="ct")
                st_sb = wp.tile([P, S], dt.float16, tag="st")

                # DMA issue order = need order: first stage of wk + first
                # stage of x chunk 0 gate the very first matmul; x chunk 1
                # ahead of the RoPE tables (first RoPE is ~28us in).
                for j in range(NSTG):
                    nc.sync.dma_start(wk_q[j][:], wkT[:, j * DQ:(j + 1) * DQ, :])
                    nc.sync.dma_start(xst[0][j][:],
                                      xTc[0, :, j * DQ:(j + 1) * DQ, :])
                for j in range(NSTG):
                    nc.sync.dma_start(xst[1][j][:],
                                      xTc[1, :, j * DQ:(j + 1) * DQ, :])
                nc.sync.dma_start(ct_sb[:], ct[:])
                nc.sync.dma_start(st_sb[:], st[:])
                for c in range(2, NCH):
                    nc.sync.dma_start(xc_t[c][:], xTc[c])
                nc.sync.dma_start(wq_sb[:], wqT[:])

                def wkslice(dc, hsl):
                    return wk_q[dc // DQ][:, dc % DQ, hsl]

                def proj_chunk_rope(wsl, outT, c):
                    """One n-chunk of a projection, dc-major with 4 open
                    PSUM banks (one per head), then RoPE per head."""
                    nsl = slice(c * CH, (c + 1) * CH)
                    banks = [psg.tile([P, CH], dt.float32, tag="gemm",
                                      bufs=8, name=f"gemm{_h}")
                             for _h in range(H_LOC)]
                    for dc in range(DCH):
                        xs = xslice(c, dc)
                        for h in range(H_LOC):
                            nc.tensor.matmul(
                                banks[h][:], wsl(dc, slice(h * HD, (h + 1) * HD)),
                                xs, start=(dc == 0), stop=(dc == DCH - 1))
                    for h in range(H_LOC):
                        # RoPE: partitions 0:64 even pairs e, 64:128 odd o:
                        #   out_e = e*c - o*s ; out_o = e*s + o*c
                        # t2s holds the sin product with halves pre-swapped so
                        # every TensorTensor's two SBUF inputs share a base
                        # partition (walrus NCC_IBIR297).
                        s16 = t2p.tile([P, CH], dt.float16, tag="s16", bufs=3)
                        nc.scalar.activation(
                            s16[:], banks[h][:],
                            mybir.ActivationFunctionType.Copy)
                        t1 = t2p.tile([P, CH], dt.float16, tag="t1", bufs=2)
                        t2s = t2p.tile([P, CH], dt.float16, tag="t2", bufs=2)
                        nc.vector.tensor_mul(out=t1[:], in0=s16[:],
                                             in1=ct_sb[:, nsl])
                        nc.vector.tensor_mul(out=t2s[0:64, :],
                                             in0=s16[64:128, :],
                                             in1=st_sb[64:128, nsl])
                        nc.vector.tensor_mul(out=t2s[64:128, :],
                                             in0=s16[0:64, :],
                                             in1=st_sb[0:64, nsl])
                        o_sl = outT[:, h, nsl]
                        nc.vector.tensor_sub(out=o_sl[0:64, :],
                                             in0=t1[0:64, :],
                                             in1=t2s[0:64, :])
                        nc.vector.tensor_add(out=o_sl[64:128, :],
                                             in0=t1[64:128, :],
                                             in1=t2s[64:128, :])

                for c in range(NCH):
                    proj_chunk_rope(wkslice, kTp, c)
                for c in range(NCH):
                    proj_chunk_rope(
                        lambda dc, hsl: wq_sb[:, dc, hsl], qTp, c)

            # ---- phase 2: v proj + scores pipeline + pv + projection -----
            with (
                tc.tile_pool(name="etp", bufs=14) as etp,
                tc.tile_pool(name="attnp", bufs=2) as attnp,
                tc.tile_pool(name="accp", bufs=1) as accp,
                tc.tile_pool(name="ytp", bufs=4) as ytp,
                tc.tile_pool(name="psc", bufs=1, space="PSUM") as psc,
            ):
                acc_of = {}
                from collections import deque
                sc_iters = deque()

                def scores_gen(b):
                    """Emit one score+exp+acc unit (2 matmuls) per yield, so
                    callers can interleave units with other TensorE work."""
                    qc, h = divmod(b, H_LOC)
                    qsl = slice(qc * CH, (qc + 1) * CH)
                    ets = []
                    acc = accp.tile([P, 2, CH], dt.float16, tag="acc", bufs=3)
                    acc_of[b] = (acc, ets)
                    for ktp in range(NT // 2):
                        ss = psc.tile([P, 2, CH], dt.float32, tag="ss", bufs=2)
                        for i in range(2):
                            kt = 2 * ktp + i
                            nc.tensor.matmul(
                                ss[:, i, :], kTp[:, h, kt * P:(kt + 1) * P],
                                qTp[:, h, qsl], start=True, stop=True)
                        et = etp.tile([P, 2, CH], dt.float16, tag="et")
                        nc.scalar.activation(
                            et[:], ss[:], mybir.ActivationFunctionType.Exp)
                        if ktp == 0:
                            nc.vector.tensor_copy(acc[:], et[:])
                        else:
                            nc.vector.tensor_add(out=acc[:], in0=acc[:],
                                                 in1=et[:])
                        ets.append(et)
                        yield

                def pump(n=1):
                    for _ in range(n):
                        while sc_iters:
                            try:
                                next(sc_iters[0])
                                break
                            except StopIteration:
                                sc_iters.popleft()

                allr_of = {}

                def emit_allred(b):
                    # denominator add + all-reduce on GpSimd (idle engine),
                    # emitted one block ahead so its latency hides under the
                    # preceding pv matmuls.
                    acc, _ = acc_of[b]
                    denom = accp.tile([P, CH], dt.float32, tag="den", bufs=2)
                    nc.vector.tensor_add(out=denom[:], in0=acc[:, 0, :],
                                         in1=acc[:, 1, :])
                    allr = accp.tile([P, CH], dt.float32, tag="allr", bufs=2)
                    nc.gpsimd.partition_all_reduce(
                        allr[:], denom[:], channels=P,
                        reduce_op=bass_isa.ReduceOp.add)
                    allr_of[b] = allr

                def pv_block(b, attn_cur):
                    qc, h = divmod(b, H_LOC)
                    boundary = (h == H_LOC - 1)  # attn gates a proj chunk
                    hsl = slice(h * HD, (h + 1) * HD)
                    acc, ets = acc_of.pop(b)
                    pv = psc.tile([P, CH], dt.float32, tag="pv", bufs=2)
                    for ktp in range(NT // 2):
                        et = ets[ktp]
                        for i in range(2):
                            kt = 2 * ktp + i
                            nc.tensor.matmul(
                                pv[:], v_sb[:, kt, hsl], et[:, i, :],
                                start=(kt == 0), stop=(kt == NT - 1))
                        if not boundary:
                            pump(1)
                    rec = accp.tile([P, CH], dt.float32, tag="rec", bufs=2)
                    if boundary:
                        # chunk-last block: its attn gates the next proj
                        # chunk, so (a) partition-reduce the denominator with
                        # accumulating ones-matmuls on the PE instead of the
                        # 4.3us GpSimd all-reduce, and (b) defer the score
                        # pumps so the reciprocal/attn-mul sit at the DVE
                        # queue head. Cuts each boundary stall ~8us -> ~2us.
                        dps = psc.tile([P, CH], dt.float32, tag="py", bufs=2)
                        nc.tensor.matmul(dps[:], ones[:], acc[:, 0, :],
                                         start=True, stop=False)
                        nc.tensor.matmul(dps[:], ones[:], acc[:, 1, :],
                                         start=False, stop=True)
                        nc.vector.reciprocal_approx_fast(rec[:], dps[:])
                    else:
                        allr = allr_of.pop(b)
                        nc.vector.reciprocal_approx_fast(rec[:], allr[:])
                    nc.vector.tensor_mul(
                        out=attn_cur[:, h, :], in0=pv[:], in1=rec[:])
                    if b + 1 < NB and (b + 1) % H_LOC != H_LOC - 1:
                        emit_allred(b + 1)
                    # boundary blocks do NOT pump: the following proj_chunk's
                    # per-group pumps keep every generator emitted in time
                    # (verified by unit-supply count), and a batch of pumped
                    # acc-adds here would sit ahead of proj's PSUM->SBUF
                    # copies in the DVE queue, stalling py-bank recycling.

                def proj_chunk(qc, attn_cur, ntls=range(NCH)):
                    for ntl in ntls:
                        nt = qc * NCH + ntl
                        for half in range(2):
                            yt = ytp.tile([P, D // 2], dt.float16, tag="yt")
                            for i in range(2):
                                oc = half * 2 + i
                                # alternate PSUM tags: the pv ring is idle
                                # during proj, so this gives 4 banks of copy
                                # slack instead of 2 -- py-bank recycling
                                # through the DVE queue was the ~5us
                                # boundary stall
                                py = psc.tile([P, CH], dt.float32,
                                              tag="py" if i == 0 else "pv",
                                              bufs=2)
                                for h in range(H_LOC):
                                    nc.tensor.matmul(
                                        py[:],
                                        attn_cur[:, h, ntl * P:(ntl + 1) * P],
                                        wo_sb[:, h, oc * CH:(oc + 1) * CH],
                                        start=(h == 0), stop=(h == H_LOC - 1))
                                ysl = yt[:, i * CH:(i + 1) * CH]
                                # split PSUM->SBUF copies across ScalarE and
                                # DVE so neither queue delays the score exps
                                if half == 0:
                                    nc.scalar.activation(
                                        ysl, py[:],
                                        mybir.ActivationFunctionType.Copy)
                                else:
                                    nc.vector.tensor_copy(ysl, py[:])
                            nc.sync.dma_start(
                                y3[:, nt, half * D // 2:(half + 1) * D // 2],
                                yt[:])
                            # keep score exp/acc units flowing between proj
                            # groups -- a burst of copies ahead of the exps
                            # stalls the next chunk boundary by ~10us
                            pump(1)

                # ---- v projection (x already resident), with the first two
                # score blocks pumped between v PSUM groups -----------------
                with tc.tile_pool(name="vwp", bufs=1) as vwp:
                    wv_sb = vwp.tile([P, DCH, F], dt.float16, tag="wv")
                    nc.sync.dma_start(wv_sb[:], wvT[:])
                    nc.sync.dma_start(wo_sb[:], woT[:])
                    sc_iters.append(scores_gen(0))
                    sc_iters.append(scores_gen(1))
                    for nchunk in range(NCH):
                        for nt in range(NCH):
                            ps = psc.tile([P, CH], dt.float32, tag="pv",
                                          bufs=2)
                            for dc in range(DCH):
                                nc.tensor.matmul(
                                    ps[:],
                                    xslice(nchunk, dc,
                                           slice(nt * P, (nt + 1) * P)),
                                    wv_sb[:, dc, :],
                                    start=(dc == 0), stop=(dc == DCH - 1))
                            nc.scalar.activation(
                                v_sb[:, nchunk * NCH + nt, :], ps[:],
                                mybir.ActivationFunctionType.Copy)
                            pump(1)

                    # steady state: [pv(b) | scores(b+2) units | proj(qc-1)]
                    attn_hist = {}
                    emit_allred(0)
                    for b in range(NB):
                        qc = b // H_LOC
                        if b % H_LOC == 0:
                            attn_hist[qc] = attnp.tile(
                                [P, H_LOC, CH], dt.float16, tag="attn",
                                name=f"attn_{qc}")
                        if b + 2 < NB:
                            sc_iters.append(scores_gen(b + 2))
                        pv_block(b, attn_hist[qc])
                        if b % H_LOC == 0 and b > 0:
                            proj_chunk(qc - 1, attn_hist.pop(qc - 1))
                    pump(100)
                    proj_chunk(NCH - 1, attn_hist.pop(NCH - 1))

    nc.compile()
    return nc


_NC_CACHE = None


def _get_program():
    global _NC_CACHE
    if _NC_CACHE is None:
        _NC_CACHE = _build_program()
    return _NC_CACHE


def _rope_tables():
    scale = np.arange(0, HD, 2, dtype=np.float32) / HD
    inv_freq = 1.0 / (10000.0 ** scale)                 # [64]
    t = np.arange(S, dtype=np.float32)
    ang = np.outer(t, inv_freq)                         # [S, 64]
    cos = np.cos(ang).T.astype(np.float32)              # [64, S]
    sin = np.sin(ang).T.astype(np.float32)
    stk = lambda a: np.ascontiguousarray(
        np.concatenate([a, a], axis=0)).astype(_F16)    # [128, S]
    return stk(cos), stk(sin)


def prepare_in_maps(x, wq, wk, wv, wo):
    x = np.asarray(x, dtype=np.float32)
    wq = np.asarray(wq, dtype=np.float32) * np.float32(1.0 / np.sqrt(HD))
    wk = np.asarray(wk, dtype=np.float32)
    wv = np.asarray(wv, dtype=np.float32)
    wo = np.asarray(wo, dtype=np.float32)

    ct_t, st_t = _rope_tables()

    # even/odd RoPE permutation of rows within each head
    perm = np.concatenate([np.arange(0, HD, 2), np.arange(1, HD, 2)])

    # [NCH, P, DCH, CH]: per-partition-contiguous x chunks
    xTc = [np.ascontiguousarray(
        x[b].T.reshape(DCH, P, NCH, CH).transpose(2, 1, 0, 3)).astype(_F16)
        for b in range(B)]

    in_maps = []
    for c in range(N_CORES):
        b, hg = divmod(c, H_LOC)
        heads = np.arange(hg * H_LOC, (hg + 1) * H_LOC)
        rows_qk = (heads[:, None] * HD + perm[None, :]).reshape(-1)  # [512]
        rows_nat = np.arange(hg * F, (hg + 1) * F)
        def pmaj(wT, groups):  # [D_in, F] -> [P, groups, F]
            return np.ascontiguousarray(
                wT.reshape(groups, P, wT.shape[1]).transpose(1, 0, 2)
            ).astype(_F16)
        in_maps.append({
            "xTc": xTc[b],
            "wqT": pmaj(wq[rows_qk].T, DCH),
            "wkT": pmaj(wk[rows_qk].T, DCH),
            "wvT": pmaj(wv[rows_nat].T, DCH),
            "woT": pmaj(wo[:, rows_nat].T, H_LOC),
            "ct": ct_t, "st": st_t,
        })
    return in_maps


def combine_results(results):
    out = np.zeros((B, S, D), dtype=np.float32)
    for c, r in enumerate(results):
        out[c // H_LOC] += r["y"].astype(np.float32)
    return out


def kernel(x, wq, wk, wv, wo):
    from concourse.bass_utils import run_bass_kernel_spmd

    nc = _get_program()
    in_maps = prepare_in_maps(x, wq, wk, wv, wo)
    res = run_bass_kernel_spmd(nc, in_maps, core_ids=list(range(N_CORES)))
    return combine_results(res.results)


if __name__ == "__main__":
    rng = np.random.default_rng(0)
    ins = {
        "x": rng.standard_normal((B, S, D), dtype=np.float32),
        "wq": rng.standard_normal((D, D), dtype=np.float32) / np.sqrt(D),
        "wk": rng.standard_normal((D, D), dtype=np.float32) / np.sqrt(D),
        "wv": rng.standard_normal((D, D), dtype=np.float32) / np.sqrt(D),
        "wo": rng.standard_normal((D, D), dtype=np.float32) / np.sqrt(D),
    }
    out = kernel(**ins)
    print("out", out.shape, out.dtype, np.abs(out).max())


# revision 19
# speedup vs baseline: 1.1945x; 1.0028x over previous
"""Multi-head attention (B=2, S=2048, D=2048, H=16, hd=128) on 8 TRN2 NeuronCores.

Sharding: data-parallel over batch (2) x tensor-parallel over head groups (4).
Core c handles batch c//4 and heads [4*(c%4), 4*(c%4)+4). Each core computes
q/k/v projections for its 512 features, RoPE, full attention over S for its 4
heads, and a partial output projection y_partial = attn_local @ wo[:, cols].T.
Host sums the 4 partials per batch (no on-chip collectives).

All matmuls run in f16 with fp32 PSUM accumulation. The 1/sqrt(hd) score
scale is folded into wq host-side. RoPE pairs are split even/odd across the
partition dim by permuting wq/wk rows host-side. Scores are computed
transposed ([k, q]) so softmax(exp)@V needs no on-chip transposes.

Trace-driven optimizations vs the original pipeline (457us -> ~394us):
- x is DMA'd once into persistent SBUF (k/q/v all reuse it); wk and x
  chunks 0-1 are staged in 2-dc tiles so the first k matmul starts ~11us
  in (runtime preamble + 1MB DMA) instead of ~20us.
- k/q projections run dc-major (4 open PSUM banks, one per head), so the
  first matmul only needs the first stage of wk and of x chunk 0.
- RoPE: one ScalarE PSUM->SBUF f16 copy per group, then DVE math on f16
  SBUF with the sin-product halves pre-swapped (walrus requires equal
  base partitions for SBUF+SBUF TensorTensor inputs).
- Chunk-last blocks (attn gates the next output-projection chunk):
  the softmax denominator partition-reduce runs as accumulating
  ones-matmuls on the PE (~0.4us) instead of the 4.3us GpSimd
  all-reduce, and the score pumps are deferred so reciprocal/attn-mul
  sit at the DVE queue head. Other blocks keep the GpSimd all-reduce
  (fully hidden under pv matmuls).
- proj_chunk pumps one score exp/acc unit per PSUM group: a burst of
  PSUM->SBUF copies enqueued ahead of the exps otherwise stalls the
  next chunk boundary ~10us and drops the PE out of its top p-state
  (matmul spacing degrades 216ns -> 259ns kernel-wide).
- Output-projection copies split ScalarE/DVE; y is f16 in DRAM (host
  accumulates in f32).
"""

import numpy as np

B = 2
S = 2048
D = 2048
H = 16
HD = 128
P = 128
N_CORES = 8
H_LOC = 4          # heads per core
F = H_LOC * HD     # local features = 512
NCH = 4            # n-chunks of 512 over S
CH = S // NCH      # 512
DCH = D // P       # 16 contraction chunks
NT = S // P        # 16 row tiles
DQ = 2             # dc-stage size (8 stages of 2 dc each)

_F16 = np.float16


def _build_program():
    import concourse.bass_isa as bass_isa
    import concourse.mybir as mybir
    import concourse.tile as tile
    from concourse import bacc

    dt = mybir.dt
    nc = bacc.Bacc("TRN2", target_bir_lowering=False, debug=False,
                   num_devices=N_CORES)

    # partition-major layouts so every DMA reads >=2KB contiguous per line
    xTc = nc.dram_tensor("xTc", [NCH, P, DCH, CH], dt.float16,
                         kind="ExternalInput").ap()
    wqT = nc.dram_tensor("wqT", [P, DCH, F], dt.float16,
                         kind="ExternalInput").ap()
    wkT = nc.dram_tensor("wkT", [P, DCH, F], dt.float16,
                         kind="ExternalInput").ap()
    wvT = nc.dram_tensor("wvT", [P, DCH, F], dt.float16,
                         kind="ExternalInput").ap()
    woT = nc.dram_tensor("woT", [P, H_LOC, D], dt.float16,
                         kind="ExternalInput").ap()
    # stacked RoPE tables: [cos;cos] and [sin;sin]
    ct = nc.dram_tensor("ct", [P, S], dt.float16, kind="ExternalInput").ap()
    st = nc.dram_tensor("st", [P, S], dt.float16, kind="ExternalInput").ap()
    y = nc.dram_tensor("y", [S, D], dt.float16, kind="ExternalOutput").ap()

    y3 = y.rearrange("(o p) n -> p o n", p=P)        # [128, 16, 2048]

    NB = NCH * H_LOC  # 16 attention blocks, b = qc*4 + h

    with tile.TileContext(nc) as tc:
        with tc.tile_pool(name="persist", bufs=1) as pp:
            qTp = pp.tile([P, H_LOC, S], dt.float16, tag="qTp")
            kTp = pp.tile([P, H_LOC, S], dt.float16, tag="kTp")
            v_sb = pp.tile([P, NT, F], dt.float16, tag="v")
            wo_sb = pp.tile([P, H_LOC, D], dt.float16, tag="wo")
            ones = pp.tile([P, P], dt.float16, tag="ones")
            # x resident for the whole kernel: chunks 0-1 as staged tiles
            # (fine-grained DMA deps so k-proj never waits), chunks 2-3 whole.
            NSTG = DCH // DQ
            xst = [[pp.tile([P, DQ, CH], dt.float16, tag=f"x{c}q{j}",
                            name=f"x{c}q{j}") for j in range(NSTG)]
                   for c in range(2)]
            xc_t = [None, None] + [
                pp.tile([P, DCH, CH], dt.float16, tag=f"xc{c}",
                        name=f"xc{c}") for c in range(2, NCH)]
            nc.vector.memset(ones[:], 1.0)

            def xslice(c, dc, nsl=slice(None)):
                if c < 2:
                    return xst[c][dc // DQ][:, dc % DQ, nsl]
                return xc_t[c][:, dc, nsl]

            # ---- phase 1: k and q projections + RoPE ---------------------
            with (
                tc.tile_pool(name="wp", bufs=1) as wp,
                tc.tile_pool(name="t2p", bufs=1) as t2p,
                tc.tile_pool(name="psg", bufs=1, space="PSUM") as psg,
            ):
                wk_q = [wp.tile([P, DQ, F], dt.float16, tag=f"wk{j}",
                               name=f"wk{j}") for j in range(DCH // DQ)]
                wq_sb = wp.tile([P, DCH, F], dt.float16, tag="wq")
                ct_sb = wp.tile([P, S], dt.float16, tag="ct")
                st_sb = wp.tile([P, S], dt.float16, tag="st")

                # DMA issue order = need order: first stage of wk + first
                # stage of x chunk 0 gate the very first matmul; x chunk 1
                # ahead of the RoPE tables (first RoPE is ~28us in).
                for j in range(NSTG):
                    nc.sync.dma_start(wk_q[j][:], wkT[:, j * DQ:(j + 1) * DQ, :])
                    nc.sync.dma_start(xst[0][j][:],
                                      xTc[0, :, j * DQ:(j + 1) * DQ, :])
                for j in range(NSTG):
                    nc.sync.dma_start(xst[1][j][:],
                                      xTc[1, :, j * DQ:(j + 1) * DQ, :])
                nc.sync.dma_start(ct_sb[:], ct[:])
                nc.sync.dma_start(st_sb[:], st[:])
                for c in range(2, NCH):
                    nc.sync.dma_start(xc_t[c][:], xTc[c])
                nc.sync.dma_start(wq_sb[:], wqT[:])

                def wkslice(dc, hsl):
                    return wk_q[dc // DQ][:, dc % DQ, hsl]

                def proj_chunk_rope(wsl, outT, c):
                    """One n-chunk of a projection, dc-major with 4 open
                    PSUM banks (one per head), then RoPE per head."""
                    nsl = slice(c * CH, (c + 1) * CH)
                    banks = [psg.tile([P, CH], dt.float32, tag="gemm",
                                      bufs=8, name=f"gemm{_h}")
                             for _h in range(H_LOC)]
                    for dc in range(DCH):
                        xs = xslice(c, dc)
                        for h in range(H_LOC):
                            nc.tensor.matmul(
                                banks[h][:], wsl(dc, slice(h * HD, (h + 1) * HD)),
                                xs, start=(dc == 0), stop=(dc == DCH - 1))
                    # emit all four PSUM->SBUF copies first, split across
                    # ScalarE and DVE, so the PSUM banks recycle in ~1.4us
                    # instead of serializing ~2.8us on ScalarE (this was a
                    # ~2.3us PE stall at every projection seam)
                    s16s = []
                    for h in range(H_LOC):
                        s16 = t2p.tile([P, CH], dt.float16, tag="s16", bufs=4,
                                       name=f"s16_{h}")
                        if h < 2:
                            nc.scalar.activation(
                                s16[:], banks[h][:],
                                mybir.ActivationFunctionType.Copy)
                        else:
                            nc.vector.tensor_copy(s16[:], banks[h][:])
                        s16s.append(s16)
                    for h in range(H_LOC):
                        # RoPE: partitions 0:64 even pairs e, 64:128 odd o:
                        #   out_e = e*c - o*s ; out_o = e*s + o*c
                        # t2s holds the sin product with halves pre-swapped so
                        # every TensorTensor's two SBUF inputs share a base
                        # partition (walrus NCC_IBIR297).
                        s16 = s16s[h]
                        t1 = t2p.tile([P, CH], dt.float16, tag="t1", bufs=2)
                        t2s = t2p.tile([P, CH], dt.float16, tag="t2", bufs=2)
                        nc.vector.tensor_mul(out=t1[:], in0=s16[:],
                                             in1=ct_sb[:, nsl])
                        nc.vector.tensor_mul(out=t2s[0:64, :],
                                             in0=s16[64:128, :],
                                             in1=st_sb[64:128, nsl])
                        nc.vector.tensor_mul(out=t2s[64:128, :],
                                             in0=s16[0:64, :],
                                             in1=st_sb[0:64, nsl])
                        o_sl = outT[:, h, nsl]
                        nc.vector.tensor_sub(out=o_sl[0:64, :],
                                             in0=t1[0:64, :],
                                             in1=t2s[0:64, :])
                        nc.vector.tensor_add(out=o_sl[64:128, :],
                                             in0=t1[64:128, :],
                                             in1=t2s[64:128, :])

                for c in range(NCH):
                    proj_chunk_rope(wkslice, kTp, c)
                for c in range(NCH):
                    proj_chunk_rope(
                        lambda dc, hsl: wq_sb[:, dc, hsl], qTp, c)

            # ---- phase 2: v proj + scores pipeline + pv + projection -----
            with (
                tc.tile_pool(name="etp", bufs=14) as etp,
                tc.tile_pool(name="attnp", bufs=2) as attnp,
                tc.tile_pool(name="accp", bufs=1) as accp,
                tc.tile_pool(name="ytp", bufs=4) as ytp,
                tc.tile_pool(name="psc", bufs=1, space="PSUM") as psc,
            ):
                acc_of = {}
                from collections import deque
                sc_iters = deque()

                def scores_gen(b):
                    """Emit one score+exp+acc unit (2 matmuls) per yield, so
                    callers can interleave units with other TensorE work."""
                    qc, h = divmod(b, H_LOC)
                    qsl = slice(qc * CH, (qc + 1) * CH)
                    ets = []
                    acc = accp.tile([P, 2, CH], dt.float16, tag="acc", bufs=3)
                    acc_of[b] = (acc, ets)
                    for ktp in range(NT // 2):
                        ss = psc.tile([P, 2, CH], dt.float32, tag="ss", bufs=2)
                        for i in range(2):
                            kt = 2 * ktp + i
                            nc.tensor.matmul(
                                ss[:, i, :], kTp[:, h, kt * P:(kt + 1) * P],
                                qTp[:, h, qsl], start=True, stop=True)
                        et = etp.tile([P, 2, CH], dt.float16, tag="et")
                        nc.scalar.activation(
                            et[:], ss[:], mybir.ActivationFunctionType.Exp)
                        if ktp == 0:
                            nc.vector.tensor_copy(acc[:], et[:])
                        else:
                            nc.vector.tensor_add(out=acc[:], in0=acc[:],
                                                 in1=et[:])
                        ets.append(et)
                        yield

                def pump(n=1):
                    for _ in range(n):
                        while sc_iters:
                            try:
                                next(sc_iters[0])
                                break
                            except StopIteration:
                                sc_iters.popleft()

                allr_of = {}

                def emit_allred(b):
                    # denominator add + all-reduce on GpSimd (idle engine),
                    # emitted one block ahead so its latency hides under the
                    # preceding pv matmuls.
                    acc, _ = acc_of[b]
                    denom = accp.tile([P, CH], dt.float32, tag="den", bufs=2)
                    nc.vector.tensor_add(out=denom[:], in0=acc[:, 0, :],
                                         in1=acc[:, 1, :])
                    allr = accp.tile([P, CH], dt.float32, tag="allr", bufs=2)
                    nc.gpsimd.partition_all_reduce(
                        allr[:], denom[:], channels=P,
                        reduce_op=bass_isa.ReduceOp.add)
                    allr_of[b] = allr

                def pv_block(b, attn_cur):
                    qc, h = divmod(b, H_LOC)
                    boundary = (h == H_LOC - 1)  # attn gates a proj chunk
                    hsl = slice(h * HD, (h + 1) * HD)
                    acc, ets = acc_of.pop(b)
                    pv = psc.tile([P, CH], dt.float32, tag="pv", bufs=2)
                    for ktp in range(NT // 2):
                        et = ets[ktp]
                        for i in range(2):
                            kt = 2 * ktp + i
                            nc.tensor.matmul(
                                pv[:], v_sb[:, kt, hsl], et[:, i, :],
                                start=(kt == 0), stop=(kt == NT - 1))
                        if not boundary:
                            pump(1)
                    rec = accp.tile([P, CH], dt.float32, tag="rec", bufs=2)
                    if boundary:
                        # chunk-last block: its attn gates the next proj
                        # chunk, so (a) partition-reduce the denominator with
                        # accumulating ones-matmuls on the PE instead of the
                        # 4.3us GpSimd all-reduce, and (b) defer the score
                        # pumps so the reciprocal/attn-mul sit at the DVE
                        # queue head. Cuts each boundary stall ~8us -> ~2us.
                        dps = psc.tile([P, CH], dt.float32, tag="py", bufs=2)
                        nc.tensor.matmul(dps[:], ones[:], acc[:, 0, :],
                                         start=True, stop=False)
                        nc.tensor.matmul(dps[:], ones[:], acc[:, 1, :],
                                         start=False, stop=True)
                        nc.vector.reciprocal_approx_fast(rec[:], dps[:])
                    else:
                        allr = allr_of.pop(b)
                        nc.vector.reciprocal_approx_fast(rec[:], allr[:])
                    nc.vector.tensor_mul(
                        out=attn_cur[:, h, :], in0=pv[:], in1=rec[:])
                    if b + 1 < NB and (b + 1) % H_LOC != H_LOC - 1:
                        emit_allred(b + 1)
                    # boundary blocks do NOT pump: the following proj_chunk's
                    # per-group pumps keep every generator emitted in time
                    # (verified by unit-supply count), and a batch of pumped
                    # acc-adds here would sit ahead of proj's PSUM->SBUF
                    # copies in the DVE queue, stalling py-bank recycling.

                def proj_chunk(qc, attn_cur, ntls=range(NCH)):
                    for ntl in ntls:
                        nt = qc * NCH + ntl
                        for half in range(2):
                            yt = ytp.tile([P, D // 2], dt.float16, tag="yt")
                            for i in range(2):
                                oc = half * 2 + i
                                # alternate PSUM tags: the pv ring is idle
                                # during proj, so this gives 4 banks of copy
                                # slack instead of 2 -- py-bank recycling
                                # through the DVE queue was the ~5us
                                # boundary stall
                                py = psc.tile([P, CH], dt.float32,
                                              tag="py" if i == 0 else "pv",
                                              bufs=2)
                                for h in range(H_LOC):
                                    nc.tensor.matmul(
                                        py[:],
                                        attn_cur[:, h, ntl * P:(ntl + 1) * P],
                                        wo_sb[:, h, oc * CH:(oc + 1) * CH],
                                        start=(h == 0), stop=(h == H_LOC - 1))
                                ysl = yt[:, i * CH:(i + 1) * CH]
                                # split PSUM->SBUF copies across ScalarE and
                                # DVE so neither queue delays the score exps
                                if half == 0:
                                    nc.scalar.activation(
                                        ysl, py[:],
                                        mybir.ActivationFunctionType.Copy)
                                else:
                                    nc.vector.tensor_copy(ysl, py[:])
                            nc.sync.dma_start(
                                y3[:, nt, half * D // 2:(half + 1) * D // 2],
                                yt[:])
                            # keep score exp/acc units flowing between proj
                            # groups -- a burst of copies ahead of the exps
                            # stalls the next chunk boundary by ~10us
                            pump(1)

                # ---- v projection (x already resident), with the first two
                # score blocks pumped between v PSUM groups -----------------
                with tc.tile_pool(name="vwp", bufs=1) as vwp:
                    wv_sb = vwp.tile([P, DCH, F], dt.float16, tag="wv")
                    nc.sync.dma_start(wv_sb[:], wvT[:])
                    nc.sync.dma_start(wo_sb[:], woT[:])
                    sc_iters.append(scores_gen(0))
                    sc_iters.append(scores_gen(1))
                    for nchunk in range(NCH):
                        for nt in range(NCH):
                            ps = psc.tile([P, CH], dt.float32, tag="pv",
                                          bufs=2)
                            for dc in range(DCH):
                                nc.tensor.matmul(
                                    ps[:],
                                    xslice(nchunk, dc,
                                           slice(nt * P, (nt + 1) * P)),
                                    wv_sb[:, dc, :],
                                    start=(dc == 0), stop=(dc == DCH - 1))
                            nc.scalar.activation(
                                v_sb[:, nchunk * NCH + nt, :], ps[:],
                                mybir.ActivationFunctionType.Copy)
                            pump(1)

                    # steady state: [pv(b) | scores(b+2) units | proj(qc-1)]
                    attn_hist = {}
                    emit_allred(0)
                    for b in range(NB):
                        qc = b // H_LOC
                        if b % H_LOC == 0:
                            attn_hist[qc] = attnp.tile(
                                [P, H_LOC, CH], dt.float16, tag="attn",
                                name=f"attn_{qc}")
                        if b + 2 < NB:
                            sc_iters.append(scores_gen(b + 2))
                        pv_block(b, attn_hist[qc])
                        if b % H_LOC == 0 and b > 0:
                            proj_chunk(qc - 1, attn_hist.pop(qc - 1))
                    pump(100)
                    proj_chunk(NCH - 1, attn_hist.pop(NCH - 1))

    nc.compile()
    return nc


_NC_CACHE = None


def _get_program():
    global _NC_CACHE
    if _NC_CACHE is None:
        _NC_CACHE = _build_program()
    return _NC_CACHE


def _rope_tables():
    scale = np.arange(0, HD, 2, dtype=np.float32) / HD
    inv_freq = 1.0 / (10000.0 ** scale)                 # [64]
    t = np.arange(S, dtype=np.float32)
    ang = np.outer(t, inv_freq)                         # [S, 64]
    cos = np.cos(ang).T.astype(np.float32)              # [64, S]
    sin = np.sin(ang).T.astype(np.float32)
    stk = lambda a: np.ascontiguousarray(
        np.concatenate([a, a], axis=0)).astype(_F16)    # [128, S]
    return stk(cos), stk(sin)


def prepare_in_maps(x, wq, wk, wv, wo):
    x = np.asarray(x, dtype=np.float32)
    wq = np.asarray(wq, dtype=np.float32) * np.float32(1.0 / np.sqrt(HD))
    wk = np.asarray(wk, dtype=np.float32)
    wv = np.asarray(wv, dtype=np.float32)
    wo = np.asarray(wo, dtype=np.float32)

    ct_t, st_t = _rope_tables()

    # even/odd RoPE permutation of rows within each head
    perm = np.concatenate([np.arange(0, HD, 2), np.arange(1, HD, 2)])

    # [NCH, P, DCH, CH]: per-partition-contiguous x chunks
    xTc = [np.ascontiguousarray(
        x[b].T.reshape(DCH, P, NCH, CH).transpose(2, 1, 0, 3)).astype(_F16)
        for b in range(B)]

    in_maps = []
    for c in range(N_CORES):
        b, hg = divmod(c, H_LOC)
        heads = np.arange(hg * H_LOC, (hg + 1) * H_LOC)
        rows_qk = (heads[:, None] * HD + perm[None, :]).reshape(-1)  # [512]
        rows_nat = np.arange(hg * F, (hg + 1) * F)
        def pmaj(wT, groups):  # [D_in, F] -> [P, groups, F]
            return np.ascontiguousarray(
                wT.reshape(groups, P, wT.shape[1]).transpose(1, 0, 2)
            ).astype(_F16)
        in_maps.append({
            "xTc": xTc[b],
            "wqT": pmaj(wq[rows_qk].T, DCH),
            "wkT": pmaj(wk[rows_qk].T, DCH),
            "wvT": pmaj(wv[rows_nat].T, DCH),
            "woT": pmaj(wo[:, rows_nat].T, H_LOC),
            "ct": ct_t, "st": st_t,
        })
    return in_maps


def combine_results(results):
    out = np.zeros((B, S, D), dtype=np.float32)
    for c, r in enumerate(results):
        out[c // H_LOC] += r["y"].astype(np.float32)
    return out


def kernel(x, wq, wk, wv, wo):
    from concourse.bass_utils import run_bass_kernel_spmd

    nc = _get_program()
    in_maps = prepare_in_maps(x, wq, wk, wv, wo)
    res = run_bass_kernel_spmd(nc, in_maps, core_ids=list(range(N_CORES)))
    return combine_results(res.results)


if __name__ == "__main__":
    rng = np.random.default_rng(0)
    ins = {
        "x": rng.standard_normal((B, S, D), dtype=np.float32),
        "wq": rng.standard_normal((D, D), dtype=np.float32) / np.sqrt(D),
        "wk": rng.standard_normal((D, D), dtype=np.float32) / np.sqrt(D),
        "wv": rng.standard_normal((D, D), dtype=np.float32) / np.sqrt(D),
        "wo": rng.standard_normal((D, D), dtype=np.float32) / np.sqrt(D),
    }
    out = kernel(**ins)
    print("out", out.shape, out.dtype, np.abs(out).max())
